# revision 8
# baseline (speedup 1.0000x reference)
"""Trainium2 Bass kernel v4 for nn_GammaNeuronNet.

Structure:
* G_syn is streamed ONCE per step: M=2 stationary [sE_k | s_k] produces the
  num-partial (row base+0) and D-partial (row base+1) together.  G_gap
  passes are M=1 with stationary V_k accumulating straight onto the num row.
  Per step: 2 const + 32 syn (M=2) + 32 gap (M=1) passes per column-group,
  two column groups (tile_position (0,0)/(0,32)) split the 512 own outputs
  into 256-col halves, so num/D per half are complete in PSUM with no
  cross-group combines.
* The "-1" of (1 - den*dt) is folded into the c0 constant row: the D rows
  hold den*dt - 1 directly.
* Engine APs must start at 32-aligned partitions, so the D rows (partitions
  1/33) are extracted by a [0:2]-based DVE copy to SBUF, DMA'd to DRAM, and
  the per-step exchange is an f32 AllGather of [num_own(512) | D_own(512)]
  (4KB/rank) instead of V itself.
* Every core then computes the V update for ALL 4096 neurons redundantly in
  the partition-parallel xw layout [128,32] (fast: w1 = (D*-1)*V;
  V' = w1 + num) -- float32 state everywhere, DVE ops cost ~32 elems/lane.
  The s-chain is likewise redundant full-N (as in v2/v3).
* The final output is the full V from core 0 (identical on all cores).
* No keep-warm dummy matmuls by default: the next step's syn burst
  (V-independent) executes inside the AllGather window.

Layouts:
* "xw" layout: [128, 32] tile, neuron n=32p+t at partition p, col t
  (flat index == neuron id).
* s2i interleaved: [128, 64] bf16, col 2k = sE_k (num vec), col 2k+1 = s_k
  (D vec): M=2 stationary for k-tile k is s2i[:, 2k:2k+2].
* AllGather payload per rank: [numA(256) | numB(256) | DA(256) | DB(256)]
  f32 == [num_own(512) | D_own(512)]; gathered buffer is rank-major, so
  num[n] sits at flat 1024r+j and D[n] at 1024r+512+j for n = 512r+j.
"""

import os
import numpy as np
import ml_dtypes

N = 4096
NCORES = 8
ROWS = N // NCORES            # 512 matrix rows per core
HALF = ROWS // 2              # 256 free columns per PE column group
KTM = N // 128                # 32 k-tiles per matrix
KT = 2 * KTM                  # 64 merged k-tiles (G_syn then G_gap)
BETA, V_TH, A_R, A_D = 0.125, -15.0, 1.0, 5.0

NDUMMY = int(os.environ.get("GAMMA_NDUMMY", "30"))   # keep-warm dummy MMs/step

_cache = {}
last_results = None


def _n_steps(timestep, runtime):
    t, n = 0.0, 0
    while t < runtime:
        t += timestep
        n += 1
    return n


def _build(n_steps: int, dt: float, fast: bool, ndummy: int):
    import concourse.bacc as bacc
    import concourse.mybir as mybir
    import concourse.tile as tile

    f32 = mybir.dt.float32
    bf16 = mybir.dt.bfloat16

    nc = bacc.Bacc("TRN2", target_bir_lowering=False, debug=False,
                   num_devices=NCORES)

    w_d = nc.dram_tensor("w_in", [128, KT * ROWS], bf16, kind="ExternalInput")
    s2_d = nc.dram_tensor("s2_0", [128, 64], bf16, kind="ExternalInput")
    vg_d = nc.dram_tensor("vg0", [128, 32], bf16, kind="ExternalInput")
    vf_d = nc.dram_tensor("vf0", [128, 32], f32, kind="ExternalInput")
    s0_d = nc.dram_tensor("s0", [128, 32], f32, kind="ExternalInput")
    esyn_d = nc.dram_tensor("esyn", [128, 32], f32, kind="ExternalInput")
    cst_d = nc.dram_tensor("cst", [128, 512], bf16, kind="ExternalInput")
    idl_d = nc.dram_tensor("idl", [128, 2], bf16, kind="ExternalInput")
    vout_d = nc.dram_tensor("v_out", [128, 32], f32, kind="ExternalOutput")

    rg = [list(range(NCORES))]
    Sigmoid = mybir.ActivationFunctionType.Sigmoid
    Copy = mybir.ActivationFunctionType.Copy
    Alu = mybir.AluOpType

    ar_dt = float(A_R) * dt
    c1 = 1.0 - float(A_D) * dt
    sig_scale = float(BETA)
    sig_bias = -float(BETA) * float(V_TH)
    inv_dt = 1.0 / dt

    with tile.TileContext(nc) as tc:
        with (
            tc.tile_pool(name="const", bufs=1) as constp,
            tc.tile_pool(name="wpool", bufs=1) as wp,
            tc.tile_pool(name="state", bufs=2) as stp,
            tc.tile_pool(name="ew", bufs=2) as ewp,
            tc.tile_pool(name="mm", bufs=2, space="PSUM") as mmp,
            tc.tile_pool(name="dum", bufs=1, space="PSUM") as dump,
            tc.tile_pool(name="dram", bufs=2, space="DRAM") as dramp,
        ):
            w_sb = wp.tile([128, KT * ROWS], bf16)
            nc.sync.dma_start(w_sb[:], w_d[:])
            esyn_sb = constp.tile([128, 32], f32)
            nc.sync.dma_start(esyn_sb[:], esyn_d[:])
            cst_sb = constp.tile([128, 512], bf16)
            nc.sync.dma_start(cst_sb[:], cst_d[:])
            idl_sb = constp.tile([128, 2], bf16)
            nc.sync.dma_start(idl_sb[:], idl_d[:])
            sigb_sb = constp.tile([128, 1], f32)
            nc.vector.memset(sigb_sb[:], sig_bias)

            # persistent state double buffers
            s2b = [stp.tile([128, 64], bf16, tag="s2", name=f"s2b{j}")
                   for j in range(2)]
            gvb = [stp.tile([128, 32], bf16, tag="gv", name=f"gvb{j}")
                   for j in range(2)]
            sfb = [stp.tile([128, 32], f32, tag="sf", name=f"sfb{j}")
                   for j in range(2)]
            vfb = [stp.tile([128, 32], f32, tag="vf", name=f"vfb{j}")
                   for j in range(2)]
            ccsb = [stp.tile([2, 512], f32, tag="cc", name=f"ccsb{j}")
                    for j in range(2)]
            nc.sync.dma_start(s2b[0][:], s2_d[:])
            nc.sync.dma_start(gvb[0][:], vg_d[:])
            nc.sync.dma_start(sfb[0][:], s0_d[:])
            nc.sync.dma_start(vfb[0][:], vf_d[:])

            # PSUM accumulators, parity-buffered: rows 0-1 = group A
            # (num, D), rows 32-33 = group B.
            mmb = [mmp.tile([128, HALF], f32, tag="mm", name=f"mm{j}")
                   for j in range(2)]

            ccin_bufs = [dramp.tile([1, 1024], f32, tag="ccin",
                                    name=f"ccinb{j}") for j in range(2)]

            def syn_burst(i):
                """const + 32 M=2 syn passes for step i into mmb[i%2]."""
                mm = mmb[i % 2]
                s2 = s2b[i % 2]
                nc.tensor.matmul(mm[0:2, :], idl_sb[:, 0:2], cst_sb[:, 0:HALF],
                                 start=True, stop=False, tile_position=(0, 0))
                nc.tensor.matmul(mm[32:34, :], idl_sb[:, 0:2],
                                 cst_sb[:, HALF:2 * HALF],
                                 start=True, stop=False, tile_position=(0, 32))
                for k in range(KTM):
                    w0 = k * ROWS
                    nc.tensor.matmul(
                        mm[0:2, :], s2[:, 2 * k:2 * k + 2],
                        w_sb[:, w0:w0 + HALF],
                        start=False, stop=False, tile_position=(0, 0))
                    nc.tensor.matmul(
                        mm[32:34, :], s2[:, 2 * k:2 * k + 2],
                        w_sb[:, w0 + HALF:w0 + ROWS],
                        start=False, stop=False, tile_position=(0, 32))

            def gap_burst(i):
                """32 M=1 gap passes for step i onto the num rows; the final
                pass also stops the D rows via a 2-row out AP with a zero
                second stationary column."""
                mm = mmb[i % 2]
                gv = gvb[i % 2]
                for k in range(KTM):
                    w0 = (KTM + k) * ROWS
                    last = k == KTM - 1
                    nc.tensor.matmul(
                        mm[0:1, :], gv[:, k:k + 1],
                        w_sb[:, w0:w0 + HALF],
                        start=False, stop=last, tile_position=(0, 0))
                    nc.tensor.matmul(
                        mm[32:33, :], gv[:, k:k + 1],
                        w_sb[:, w0 + HALF:w0 + ROWS],
                        start=False, stop=last, tile_position=(0, 32))

            for i in range(n_steps):
                last = i == n_steps - 1
                mm = mmb[i % 2]
                if i == 0:
                    syn_burst(0)

                gap_burst(i)

                # ---- s chain for step i+1 (redundant full-N, xw layout,
                #      reads vfb[i%2]=V_i f32; issued before the tail so its
                #      DVE ops run inside the gap-burst window)
                if not last:
                    sig = ewp.tile([128, 32], f32, tag="sig")
                    u = ewp.tile([128, 32], f32, tag="u")
                    w_ = ewp.tile([128, 32], f32, tag="w")
                    p2 = ewp.tile([128, 32], f32, tag="p2")
                    snew = sfb[(i + 1) % 2]
                    s2n = s2b[(i + 1) % 2]
                    nc.scalar.activation(sig[:], vfb[i % 2][:], Sigmoid,
                                         bias=sigb_sb[:, 0:1], scale=sig_scale)
                    nc.scalar.activation(u[:], sig[:], Copy, bias=0.0,
                                         scale=ar_dt)
                    nc.scalar.activation(w_[:], u[:], Copy, bias=c1,
                                         scale=-1.0)
                    nc.vector.tensor_mul(p2[:], sfb[i % 2][:], w_[:])
                    nc.vector.tensor_add(snew[:], p2[:], u[:])
                    nc.vector.tensor_mul(s2n[:, 0:64:2], snew[:], esyn_sb[:])
                    nc.vector.tensor_copy(s2n[:, 1:64:2], snew[:])

                # ---- extract num/D rows: [0:2]-aligned copies PSUM->SBUF,
                #      then strided DMAs to the exchange buffer.  ccin layout
                #      per own 32-neuron group g: [num_g(32) | D_g(32)], so
                #      the gathered [128,64] buffer has num at cols 0-31 and
                #      D at cols 32-63 of every partition (xw rows).
                cc = ccsb[i % 2]
                nc.vector.tensor_copy(cc[0:2, 0:HALF], mm[0:2, :])
                nc.scalar.activation(cc[0:2, HALF:ROWS], mm[32:34, :], Copy,
                                     bias=0.0, scale=1.0)
                ccin = ccin_bufs[i % 2]
                ccin3 = ccin[:].rearrange("o (g w t) -> o g w t", g=16, w=2)
                nc.sync.dma_start(ccin3[:, :, 0, :],
                                  cc[0:1, :].rearrange("o (g t) -> o g t",
                                                       g=16))
                nc.scalar.dma_start(ccin3[:, :, 1, :],
                                    cc[1:2, :].rearrange("o (g t) -> o g t",
                                                         g=16))
                ccout = nc.dram_tensor(f"ccout{i}", [128, 64], f32,
                                       addr_space="Shared")
                nc.gpsimd.collective_compute(
                    "AllGather", mybir.AluOpType.bypass, replica_groups=rg,
                    ins=[ccin[:].opt()], outs=[ccout[:].opt()])

                # ---- gathered buffer is already xw: one contiguous load
                nd = ewp.tile([128, 64], f32, tag="nd")
                nc.sync.dma_start(nd[:], ccout[:])
                num_xw = nd[:, 0:32]
                den_xw = nd[:, 32:64]

                # ---- V update, redundant full-N in xw layout
                vold = vfb[i % 2]
                vnew = vfb[(i + 1) % 2]
                w1 = ewp.tile([128, 32], f32, tag="w1")
                if fast:
                    # D = den*dt - 1:  V' = (D * -1) * V + num
                    nc.vector.scalar_tensor_tensor(
                        w1[:], den_xw, -1.0, vold[:],
                        op0=Alu.mult, op1=Alu.mult)
                    nc.vector.tensor_add(vnew[:], w1[:], num_xw)
                    if not last:
                        # gap stationary for step i+1 (parallel engine)
                        nc.gpsimd.tensor_add(gvb[(i + 1) % 2][:], w1[:],
                                             num_xw)
                else:
                    # vstep = (num - V*den) * min(dt, 1/den), D = den
                    m_ = ewp.tile([128, 32], f32, tag="m_")
                    r_ = ewp.tile([128, 32], f32, tag="r_")
                    t_ = ewp.tile([128, 32], f32, tag="t_")
                    dv = ewp.tile([128, 32], f32, tag="dv")
                    nc.vector.tensor_scalar_max(m_[:], den_xw, inv_dt)
                    nc.vector.reciprocal(r_[:], m_[:])
                    nc.vector.scalar_tensor_tensor(
                        t_[:], den_xw, -1.0, vold[:],
                        op0=Alu.mult, op1=Alu.mult)
                    nc.vector.tensor_add(t_[:], t_[:], num_xw)
                    nc.vector.tensor_mul(dv[:], t_[:], r_[:])
                    nc.vector.tensor_add(vnew[:], vold[:], dv[:])
                    if not last:
                        nc.gpsimd.tensor_add(gvb[(i + 1) % 2][:], vold[:],
                                             dv[:])
                if last:
                    nc.sync.dma_start(vout_d[:], vnew[:])
                    break

                # ---- next step's syn burst (fills the AllGather window)
                syn_burst(i + 1)

                # ---- optional keep-warm dummy matmuls
                if ndummy:
                    dps = dump.tile([2, HALF], f32, tag="dummy")
                    for _ in range(ndummy):
                        nc.tensor.matmul(dps[0:2, :], idl_sb[:, 0:2],
                                         cst_sb[:, 0:HALF], start=True,
                                         stop=True, tile_position=(0, 0),
                                         skip_group_check=True)

    nc.compile()
    return nc


def _prep(input_V, G_leak, E_leak, G_syn, E_syn, G_gap, dt, fast):
    iv = np.asarray(input_V, np.float32).reshape(-1)
    G_leak = np.asarray(G_leak, np.float32)
    E_leak = np.asarray(E_leak, np.float32)
    G_syn = np.asarray(G_syn, np.float32)
    E_syn = np.asarray(E_syn, np.float32)
    G_gap = np.asarray(G_gap, np.float32)
    in_len = iv.shape[0]

    in_avg = np.float32(iv.mean(dtype=np.float32))
    V0 = np.concatenate([iv, np.full(N - in_len, in_avg, np.float32)])
    x = (BETA * (V0 - V_TH)).astype(np.float32)
    sig = (1.0 / (1.0 + np.exp(-x, dtype=np.float32))).astype(np.float32)
    s0 = (A_R * sig / (A_R * sig + A_D)).astype(np.float32)
    sE0 = (s0 * E_syn).astype(np.float32)
    co_gap = G_gap.sum(axis=1, dtype=np.float32)
    gle_full = (G_leak * E_leak).astype(np.float32)
    c0_full = (G_leak + co_gap).astype(np.float32)

    wscale = np.float32(dt) if fast else np.float32(1.0)
    Gs16 = (G_syn * wscale).astype(ml_dtypes.bfloat16)
    Gg16 = (G_gap * wscale).astype(ml_dtypes.bfloat16)
    gle_full = gle_full * wscale
    c0_full = c0_full * wscale
    if fast:
        # fold the "-1" of (1 - den*dt) into the D constant row
        c0_full = c0_full - np.float32(1.0)

    def hilo(v):
        hi = v.astype(ml_dtypes.bfloat16)
        lo = (v - hi.astype(np.float32)).astype(ml_dtypes.bfloat16)
        return hi, lo

    def xw(v):
        # full-N vector -> [128, 32] xw layout (neuron 32p+t at (p, t))
        return np.ascontiguousarray(v.reshape(128, 32))

    # idl stationary: col 0 selects the num-const rows (gle hi/lo at
    # partitions 0,1), col 1 the D-const rows (c0 hi/lo at partitions 2,3).
    idl = np.zeros((128, 2), ml_dtypes.bfloat16)
    idl[0, 0] = idl[1, 0] = 1.0
    idl[2, 1] = idl[3, 1] = 1.0

    # s2 interleaved: col 2k = sE_k (num vec), col 2k+1 = s_k (D vec)
    s2_0 = np.zeros((128, 64), ml_dtypes.bfloat16)
    s2_0[:, 0::2] = xw(sE0)
    s2_0[:, 1::2] = xw(s0)

    in_maps = []
    for c in range(NCORES):
        rows = slice(c * ROWS, (c + 1) * ROWS)
        A_s = Gs16[rows, :].reshape(ROWS, 128, 32)   # [n, p, t], k = 32p + t
        A_g = Gg16[rows, :].reshape(ROWS, 128, 32)
        Ws = np.transpose(A_s, (1, 2, 0))            # [p, t, n]
        Wg = np.transpose(A_g, (1, 2, 0))
        W = np.ascontiguousarray(
            np.concatenate([Ws, Wg], axis=1)
        ).reshape(128, KT * ROWS)

        glehi, glelo = hilo(gle_full[rows])
        c0hi, c0lo = hilo(c0_full[rows])
        cst = np.zeros((128, 512), ml_dtypes.bfloat16)
        cst[0] = glehi
        cst[1] = glelo
        cst[2] = c0hi
        cst[3] = c0lo

        in_maps.append({
            "w_in": W,
            "s2_0": s2_0,
            "vg0": xw(V0).astype(ml_dtypes.bfloat16),
            "vf0": xw(V0),
            "s0": xw(s0),
            "esyn": xw(E_syn),
            "cst": cst,
            "idl": idl,
        })
    return in_maps, in_len


def kernel(input_V, G_leak, E_leak, G_syn, E_syn, G_gap, timestep, runtime):
    global last_results
    from concourse.bass_utils import run_bass_kernel_spmd

    dt = float(np.asarray(timestep))
    rt = float(np.asarray(runtime))
    n_steps = _n_steps(dt, rt)

    # den*dt stays below 1 iff leak+gap+syn conductances are small enough;
    # then clip(dV*dt, +-|V_inf-V|) == dV*dt exactly and the kernel can skip
    # the reciprocal/min entirely (weights prescaled by dt instead).
    G_leak_a = np.asarray(G_leak, np.float32)
    G_syn_a = np.asarray(G_syn, np.float32)
    G_gap_a = np.asarray(G_gap, np.float32)
    s_bound = max(0.21, float(A_R / (A_R + A_D)) + 0.05)
    den_bound = float((G_leak_a + G_gap_a.sum(1) +
                       G_syn_a.sum(1) * s_bound).max()) * dt
    fast = den_bound < 0.95

    key = (n_steps, dt, fast, NDUMMY)
    if key not in _cache:
        _cache[key] = _build(n_steps, dt, fast, NDUMMY)
    nc = _cache[key]

    in_maps, in_len = _prep(input_V, G_leak, E_leak, G_syn, E_syn, G_gap,
                            dt, fast)
    trace = os.environ.get("GAMMA_TRACE", "0") == "1"
    res = run_bass_kernel_spmd(
        nc, in_maps, core_ids=list(range(NCORES)), trace=trace
    )
    last_results = res

    # every core computes the identical full V; take core 0's
    V = np.asarray(res.results[0]["v_out"]).reshape(N).astype(np.float32)
    V[in_len:] = 0.0
    return V
